# revision 18
# baseline (speedup 1.0000x reference)
"""DGCNN feature extractor on 8 Trainium2 NeuronCores (Bass/Tile).

Strategy: data-parallel over batch B=8 (one sample per core).
Per layer (edge-conv):
  - scores s[n,m] = <x_n, x_m> - |x_m|^2/2 via PE matmul with an appended
    constant row (rank-equivalent to the reference's -pairwise-distance);
    fp32 throughout -- reduced-precision scores (bf16/tf32) fail the 2e-2
    tolerance because KNN selection feeds back through BN into later layers
  - scores land in PSUM halves [128,2048]; the Activation engine copies
    each half to a full-width SBUF row buffer, freeing PSUM so the next
    tile's matmuls overlap the current tile's DVE scans
  - exact-ish top-16 per row: chunked max8 (8 chunks of 512) -> top-16 of
    candidates -> max_index over the SBUF score row for global indices
  - conv decomposes as y[o,n,k] = u[o,n] + v[o, idx[n,k]] with
    u = (A-B)x, v = Bx (W = [A|B] center/diff split, host-prepped); 16
    per-k indirect DMAs gather v rows (vector-indirect SWDGE supports one
    offset per partition); v^2 is recomputed on the idle Activation
    engine and all k-folds are single-pass strided X-reduces on DVE
  - BN uses batch stats: per-core partial sums all-reduced across the 8
    cores (collective AllReduce), then
    x_next = relu(a*(u + max_k v) + b) since ReLU(LeakyReLU(z)) = ReLU(z)
    and the BN scale is positive, so max over k commutes with the affine.
    The (u+D) transpose runs while the AllReduce is in flight.
Final: per-channel max over points, concat 32+32+64, FC on device, host
stacks the 8 per-core [64] outputs.
"""
import numpy as np

B, C0, N, KNB = 8, 3, 4096, 16
O1, O2, O3 = 32, 32, 64
NCORES = 8
EPS = 1e-5
NTOT = float(B * N * KNB)
NT = N // 128          # 32 point-tiles per layer
NCH = 8                # score chunks per row (4096/512)
CHK = N // NCH         # 256
NEG = -3.0e38

_cache: dict = {}


def _build(sim_single=False, use_collective=True, ssb_bufs=2):
    import concourse.bacc as bacc
    import concourse.bass as bass
    import concourse.mybir as mybir
    import concourse.tile as tile
    from concourse.masks import make_identity

    f32 = mybir.dt.float32
    u32 = mybir.dt.uint32
    AO = mybir.AluOpType
    AF = mybir.ActivationFunctionType

    nc = bacc.Bacc("TRN2", target_bir_lowering=False, debug=False,
                   num_devices=1 if sim_single else NCORES)

    # ---- I/O ----
    x_in = nc.dram_tensor("x", [C0, N], f32, kind="ExternalInput")
    wuv_in = [None,
              nc.dram_tensor("wuv1", [C0, 2 * O1], f32, kind="ExternalInput"),
              nc.dram_tensor("wuv2", [O1, 2 * O2], f32, kind="ExternalInput"),
              nc.dram_tensor("wuv3", [O2, 2 * O3], f32, kind="ExternalInput")]
    gb_in = [None,
             nc.dram_tensor("gb1", [O1, 2], f32, kind="ExternalInput"),
             nc.dram_tensor("gb2", [O2, 2], f32, kind="ExternalInput"),
             nc.dram_tensor("gb3", [O3, 2], f32, kind="ExternalInput")]
    wfct_in = nc.dram_tensor("wfct", [128, 64], f32, kind="ExternalInput")
    bfc_in = nc.dram_tensor("bfc", [1, 64], f32, kind="ExternalInput")
    out_d = nc.dram_tensor("out", [1, 64], f32, kind="ExternalOutput")

    # ---- internal DRAM ----
    vtab = [None,
            nc.dram_tensor("vtab1", [N, O1], f32),
            nc.dram_tensor("vtab2", [N, O2], f32),
            nc.dram_tensor("vtab3", [N, O3], f32)]
    cc_in = [None] + [nc.dram_tensor(f"ccin{l}", [o, 2], f32)
                      for l, o in ((1, O1), (2, O2), (3, O3))]
    cc_out = [None] + [nc.dram_tensor(f"ccout{l}", [o, 2], f32,
                                      addr_space="Shared")
                       for l, o in ((1, O1), (2, O2), (3, O3))]

    with tile.TileContext(nc) as tc:
        with (
            tc.tile_pool(name="big", bufs=2) as bigp,        # xq/xk generations
            tc.tile_pool(name="lay", bufs=1) as layp,        # per-layer buffers
            tc.tile_pool(name="work", bufs=3) as workp,      # small per-tile tiles
            tc.tile_pool(name="ssb", bufs=ssb_bufs) as ssbp,        # SBUF score rows
            tc.tile_pool(name="gpool", bufs=3) as gp,        # gather tiles
            tc.tile_pool(name="const", bufs=1) as constp,
        ):
            ident = constp.tile([128, 128], f32)
            make_identity(nc, ident[:])
            ones128 = constp.tile([128, 1], f32)
            nc.vector.memset(ones128[:], 1.0)
            zero128 = constp.tile([128, 1], f32)
            nc.vector.memset(zero128[:], 0.0)
            eps128 = constp.tile([128, 1], f32)
            nc.vector.memset(eps128[:], EPS)
            xg = constp.tile([128, 1], f32)   # pooled channel maxes (x1|x2|x3)

            def layer(l, C, O, xq, xk, is_last):
                """xq/xk: [C+1, N] SBUF tiles, rows 0..C-1 = x, row C = aug.
                Returns next layer's (xq, xk) or None if is_last."""
                # --- phase A: aug rows, u/v matmuls, vtab ---
                wuv = constp.tile([C, 2 * O], f32, tag=f"wuv{l}")
                nc.sync.dma_start(out=wuv[:], in_=wuv_in[l].ap())
                gb = constp.tile([O, 2], f32, tag=f"gb{l}")
                nc.sync.dma_start(out=gb[:], in_=gb_in[l].ap())

                # aug rows via partition-0 staging (engine ops can't target
                # arbitrary base partitions; DMA can)
                rowst = layp.tile([1, N], f32, tag="rowst")
                nc.vector.memset(rowst[:], 1.0)
                nc.sync.dma_start(out=xq[C:C + 1, :], in_=rowst[:])
                # sq row: x^2 (chunked) then ones-matmul per 512-chunk
                onesC = constp.tile([C, 1], f32, tag=f"onesC{l}")
                nc.vector.memset(onesC[:], 1.0)
                sqst = rowst
                ubuf = layp.tile([128, NT, O], f32, tag="ubuf")
                with tc.tile_pool(name=f"psA{l}", bufs=4, space="PSUM") as psA:
                    for ch in range(8):
                        sl = slice(512 * ch, 512 * (ch + 1))
                        xsq = workp.tile([C, 512], f32, tag="xsq")
                        nc.scalar.activation(out=xsq[:], in_=xq[0:C, sl],
                                             func=AF.Square, bias=zero128[0:C, :])
                        sq_ps = psA.tile([1, 512], f32, tag="sqps")
                        nc.tensor.matmul(out=sq_ps[:], lhsT=onesC[:],
                                         rhs=xsq[:], start=True, stop=True)
                        nc.scalar.activation(out=sqst[:, sl], in_=sq_ps[:],
                                             func=AF.Copy, scale=-0.5)
                    nc.sync.dma_start(out=xk[C:C + 1, :], in_=sqst[:])
                    for t in range(NT):
                        tl = slice(128 * t, 128 * (t + 1))
                        uv_ps = psA.tile([128, 2 * O], f32, tag="uvps")
                        nc.tensor.matmul(out=uv_ps[:], lhsT=xq[0:C, tl],
                                         rhs=wuv[:], start=True, stop=True)
                        nc.scalar.activation(out=ubuf[:, t, :], in_=uv_ps[:, 0:O],
                                             func=AF.Copy)
                        vstage = workp.tile([128, O], f32, tag="vstage")
                        nc.scalar.activation(out=vstage[:], in_=uv_ps[:, O:2 * O],
                                             func=AF.Copy)
                        nc.sync.dma_start(out=vtab[l].ap()[tl, :], in_=vstage[:])

                # --- phase B: scores + topk + gather + folds ---
                Dbuf = layp.tile([128, NT, O], f32, tag="Dbuf")
                GG = layp.tile([128, NT, 2 * O], f32, tag="GG")
                with tc.tile_pool(name=f"psB{l}", bufs=2, space="PSUM") as psB:
                  for t in range(NT):
                    tl = slice(128 * t, 128 * (t + 1))
                    ssb = ssbp.tile([128, N], f32, tag="ssb")
                    for h in range(2):
                        sps = psB.tile([128, 2048], f32, tag="sps")
                        for q in range(4):
                            so = slice(512 * q, 512 * (q + 1))
                            si = slice(2048 * h + 512 * q, 2048 * h + 512 * (q + 1))
                            nc.tensor.matmul(out=sps[:, so], lhsT=xq[:, tl],
                                             rhs=xk[:, si], start=True, stop=True)
                        nc.scalar.activation(out=ssb[:, 2048 * h:2048 * (h + 1)],
                                             in_=sps[:], func=AF.Copy)
                    cand = workp.tile([128, 8 * NCH], f32, tag="cand")
                    for ch in range(NCH):
                        nc.vector.max(out=cand[:, 8 * ch:8 * ch + 8],
                                      in_=ssb[:, CHK * ch:CHK * (ch + 1)])
                    t16 = workp.tile([128, 16], f32, tag="t16")
                    cand2 = workp.tile([128, 8 * NCH], f32, tag="cand2")
                    nc.vector.max(out=t16[:, 0:8], in_=cand[:])
                    nc.vector.match_replace(out=cand2[:], in_to_replace=t16[:, 0:8],
                                            in_values=cand[:], imm_value=NEG)
                    nc.vector.max(out=t16[:, 8:16], in_=cand2[:])
                    idxs = workp.tile([128, 16], u32, tag="idxs")
                    nc.vector.max_index(out=idxs[:, 0:8], in_max=t16[:, 0:8],
                                        in_values=ssb[:])
                    nc.vector.max_index(out=idxs[:, 8:16], in_max=t16[:, 8:16],
                                        in_values=ssb[:])
                    # 16 indirect gathers (vector-indirect: 1 offset/partition)
                    g = gp.tile([128, KNB, O], f32, tag="g")
                    for k in range(KNB):
                        nc.gpsimd.indirect_dma_start(
                            out=g[:, k, :], out_offset=None, in_=vtab[l].ap(),
                            in_offset=bass.IndirectOffsetOnAxis(
                                ap=idxs[:, k:k + 1], axis=0))
                    g2 = gp.tile([128, KNB, O], f32, tag="g2")
                    nc.scalar.activation(out=g2[:], in_=g[:], func=AF.Square)
                    # single-pass strided X-reduces over k on DVE
                    gv = g[:, :, :]
                    gvs = bass.AP(gv.tensor, gv.offset,
                                  [gv.ap[0], gv.ap[2], gv.ap[1]])
                    g2v = g2[:, :, :]
                    g2s = bass.AP(g2v.tensor, g2v.offset,
                                  [g2v.ap[0], g2v.ap[2], g2v.ap[1]])
                    nc.vector.tensor_reduce(out=Dbuf[:, t, :], in_=gvs,
                                            axis=mybir.AxisListType.X, op=AO.max)
                    nc.vector.tensor_reduce(out=GG[:, t, 0:O], in_=gvs,
                                            axis=mybir.AxisListType.X, op=AO.add)
                    nc.vector.tensor_reduce(out=GG[:, t, O:2 * O], in_=g2s,
                                            axis=mybir.AxisListType.X, op=AO.add)

                # --- phase C ---
                # transposes first: PE/Act overlap Pool's fixup/fold chain
                nc.gpsimd.tensor_tensor(out=Dbuf[:], in0=Dbuf[:], in1=ubuf[:],
                                        op=AO.add)
                if is_last:
                    xnq = bigp.tile([O3 + 1, N], f32, tag="xq")
                    xnk = None
                else:
                    xnq = bigp.tile([O + 1, N], f32, tag="xq")
                    xnk = bigp.tile([O + 1, N], f32, tag="xk")
                with tc.tile_pool(name=f"psT{l}", bufs=4, space="PSUM") as psT:
                    for t in range(NT):
                        tl = slice(128 * t, 128 * (t + 1))
                        tp = psT.tile([O, 128], f32, tag="tpps")
                        nc.tensor.transpose(out=tp[:], in_=Dbuf[:, t, :],
                                            identity=ident[:])
                        nc.scalar.activation(out=xnq[0:O, tl], in_=tp[:],
                                             func=AF.Copy)
                # u-fixups: v-half += 16*u; sq-half += 2*u*Gv + 16*u^2
                tmp2 = layp.tile([128, NT, O], f32, tag="tmp2")
                nc.gpsimd.tensor_tensor(out=tmp2[:], in0=ubuf[:], in1=GG[:, :, 0:O],
                                        op=AO.mult)
                nc.gpsimd.tensor_scalar(tmp2[:], tmp2[:], 2.0, None, op0=AO.mult)
                nc.gpsimd.tensor_tensor(out=GG[:, :, O:2 * O], in0=GG[:, :, O:2 * O],
                                        in1=tmp2[:], op=AO.add)
                nc.gpsimd.tensor_tensor(out=tmp2[:], in0=ubuf[:], in1=ubuf[:],
                                        op=AO.mult)
                nc.gpsimd.tensor_scalar(tmp2[:], tmp2[:], 16.0, None, op0=AO.mult)
                nc.gpsimd.tensor_tensor(out=GG[:, :, O:2 * O], in0=GG[:, :, O:2 * O],
                                        in1=tmp2[:], op=AO.add)
                nc.gpsimd.tensor_scalar(tmp2[:], ubuf[:], 16.0, None, op0=AO.mult)
                nc.gpsimd.tensor_tensor(out=GG[:, :, 0:O], in0=GG[:, :, 0:O],
                                        in1=tmp2[:], op=AO.add)
                for hh in (16, 8, 4, 2, 1):
                    nc.gpsimd.tensor_tensor(out=GG[:, 0:hh, :], in0=GG[:, 0:hh, :],
                                            in1=GG[:, hh:2 * hh, :], op=AO.add)
                with tc.tile_pool(name=f"psR{l}", bufs=1, space="PSUM") as psR:
                    s1_ps = psR.tile([O, 1], f32, tag="s1ps")
                    s2_ps = psR.tile([O, 1], f32, tag="s2ps")
                    nc.tensor.matmul(out=s1_ps[:], lhsT=GG[:, 0, 0:O], rhs=ones128[:],
                                     start=True, stop=True)
                    nc.tensor.matmul(out=s2_ps[:], lhsT=GG[:, 0, O:2 * O], rhs=ones128[:],
                                     start=True, stop=True)
                    stg = workp.tile([O, 2], f32, tag="stg")
                    nc.vector.tensor_copy(out=stg[:, 0:1], in_=s1_ps[:])
                    nc.vector.tensor_copy(out=stg[:, 1:2], in_=s2_ps[:])
                    nc.sync.dma_start(out=cc_in[l].ap(), in_=stg[:])
                if sim_single or not use_collective:
                    nc.sync.dma_start(out=cc_out[l].ap(), in_=cc_in[l].ap())
                else:
                    nc.gpsimd.collective_compute(
                        "AllReduce", AO.add, replica_groups=[list(range(NCORES))],
                        ins=[cc_in[l].ap()], outs=[cc_out[l].ap()])

                # --- phase C ---
                # transposes first: PE/Act overlap Pool's fixup/fold chain
                nc.gpsimd.tensor_tensor(out=Dbuf[:], in0=Dbuf[:], in1=ubuf[:],
                                        op=AO.add)
                if is_last:
                    xnq = bigp.tile([O3 + 1, N], f32, tag="xq")
                    xnk = None
                else:
                    xnq = bigp.tile([O + 1, N], f32, tag="xq")
                    xnk = bigp.tile([O + 1, N], f32, tag="xk")
                with tc.tile_pool(name=f"psT{l}", bufs=4, space="PSUM") as psT:
                    for t in range(NT):
                        tl = slice(128 * t, 128 * (t + 1))
                        tp = psT.tile([O, 128], f32, tag="tpps")
                        nc.tensor.transpose(out=tp[:], in_=Dbuf[:, t, :],
                                            identity=ident[:])
                        nc.scalar.activation(out=xnq[0:O, tl], in_=tp[:],
                                             func=AF.Copy)
                # u-fixups: v-half += 16*u; sq-half += 2*u*Gv + 16*u^2
                tmp2 = layp.tile([128, NT, O], f32, tag="tmp2")
                # stats arrive: finish BN affine coefficients
                stats = workp.tile([O, 2], f32, tag="stats")
                nc.sync.dma_start(out=stats[:], in_=cc_out[l].ap())
                mean = workp.tile([O, 4], f32, tag="mean")
                nc.vector.tensor_scalar(mean[:, 0:1], stats[:, 0:1], 1.0 / NTOT,
                                        None, op0=AO.mult)
                nc.vector.tensor_scalar(mean[:, 1:2], stats[:, 1:2], 1.0 / NTOT,
                                        None, op0=AO.mult)
                # var = E2 - mean^2 ; sd = sqrt(var+eps); a = gamma/sd; b = beta - mean*a
                nc.vector.tensor_tensor(out=mean[:, 2:3], in0=mean[:, 0:1],
                                        in1=mean[:, 0:1], op=AO.mult)
                nc.vector.tensor_sub(mean[:, 1:2], mean[:, 1:2], mean[:, 2:3])
                nc.scalar.activation(out=mean[:, 1:2], in_=mean[:, 1:2],
                                     func=AF.Sqrt, bias=eps128[0:O, :])
                ab = workp.tile([O, 2], f32, tag="ab")
                nc.vector.reciprocal(out=ab[:, 0:1], in_=mean[:, 1:2])
                nc.vector.tensor_tensor(out=ab[:, 0:1], in0=ab[:, 0:1],
                                        in1=gb[:, 0:1], op=AO.mult)
                nc.vector.tensor_tensor(out=mean[:, 3:4], in0=mean[:, 0:1],
                                        in1=ab[:, 0:1], op=AO.mult)
                nc.vector.tensor_sub(ab[:, 1:2], gb[:, 1:2], mean[:, 3:4])

                # x_next = relu(a*(u+D) + b): in-place affine, then copy to xk
                nc.scalar.activation(out=xnq[0:O, :], in_=xnq[0:O, :],
                                     func=AF.Relu,
                                     bias=ab[:, 1:2], scale=ab[:, 0:1])
                if xnk is not None:
                    nc.scalar.activation(out=xnk[0:O, :], in_=xnq[0:O, :],
                                         func=AF.Copy)
                # channel max pool -> xg slice
                cm = workp.tile([O, 1], f32, tag="cm")
                nc.vector.tensor_reduce(out=cm[:], in_=xnq[0:O, :],
                                        axis=mybir.AxisListType.X, op=AO.max)
                off = {1: 0, 2: O1, 3: O1 + O2}[l]
                nc.sync.dma_start(out=xg[off:off + O, :], in_=cm[:])
                return xnq, xnk

            # layer 1 input
            xq1 = bigp.tile([C0 + 1, N], f32, tag="xq")
            xk1 = bigp.tile([C0 + 1, N], f32, tag="xk")
            nc.sync.dma_start(out=xq1[0:C0, :], in_=x_in.ap())
            nc.vector.tensor_copy(out=xk1[0:C0, :], in_=xq1[0:C0, :])

            xq2, xk2 = layer(1, C0, O1, xq1, xk1, False)
            xq3, xk3 = layer(2, O1, O2, xq2, xk2, False)
            layer(3, O2, O3, xq3, xk3, True)

            # FC: out = xg^T @ WfcT + bfc
            wfct = constp.tile([128, 64], f32)
            nc.sync.dma_start(out=wfct[:], in_=wfct_in.ap())
            bfc = constp.tile([1, 64], f32)
            nc.sync.dma_start(out=bfc[:], in_=bfc_in.ap())
            with tc.tile_pool(name="psF", bufs=1, space="PSUM") as psF:
                fc_ps = psF.tile([1, 64], f32, tag="fcps")
                nc.tensor.matmul(out=fc_ps[:], lhsT=xg[:], rhs=wfct[:],
                                 start=True, stop=True)
                ores = constp.tile([1, 64], f32)
                nc.vector.tensor_add(ores[:], fc_ps[:], bfc[:])
                nc.sync.dma_start(out=out_d.ap(), in_=ores[:])

    nc.compile()
    return nc


def _get_nc():
    if "nc" not in _cache:
        _cache["nc"] = _build()
    return _cache["nc"]


def _prep_inputs(x, W1, g1, b1, W2, g2, b2, W3, g3, b3, Wfc, bfc):
    """Host-side weight prep -> per-core input maps."""
    def wuv(W, C):
        A, Bm = W[:, :C], W[:, C:]
        return np.concatenate([(A - Bm).T, Bm.T], axis=1).astype(np.float32)

    common = {
        "wuv1": wuv(np.asarray(W1), C0),
        "wuv2": wuv(np.asarray(W2), O1),
        "wuv3": wuv(np.asarray(W3), O2),
        "gb1": np.stack([g1, b1], 1).astype(np.float32),
        "gb2": np.stack([g2, b2], 1).astype(np.float32),
        "gb3": np.stack([g3, b3], 1).astype(np.float32),
        "wfct": np.asarray(Wfc).T.copy().astype(np.float32),
        "bfc": np.asarray(bfc)[None, :].astype(np.float32),
    }
    x = np.asarray(x, dtype=np.float32)
    return [{**common, "x": np.ascontiguousarray(x[c])} for c in range(NCORES)]


def _enable_jax_cache():
    try:
        import jax
        jax.config.update("jax_compilation_cache_dir", "/tmp/jaxcache")
        jax.config.update("jax_persistent_cache_min_entry_size_bytes", -1)
        jax.config.update("jax_persistent_cache_min_compile_time_secs", 0.5)
    except Exception:
        pass


def kernel(x, W1, g1, b1, W2, g2, b2, W3, g3, b3, Wfc, bfc):
    from concourse.bass_utils import run_bass_kernel_spmd
    _enable_jax_cache()
    nc = _get_nc()
    in_maps = _prep_inputs(x, W1, g1, b1, W2, g2, b2, W3, g3, b3, Wfc, bfc)
    res = run_bass_kernel_spmd(nc, in_maps, list(range(NCORES)))
    return np.stack([res.results[c]["out"][0] for c in range(NCORES)]).astype(np.float32)
